# revision 11
# baseline (speedup 1.0000x reference)
"""Trainium2 Bass kernel for nn_MultiHeadLinearAttention (Linformer attention).

B=4, T=4096, C=1024, H=16, HS=64, K=256.
Sharding: 8 cores = batch (4) x head-group (2 groups of 8 heads).
Per core: qkv projections, low-rank kpT/vpT = k^T E / v^T E accumulated over
T, masked softmax attention over the compressed dim (exp on ScalarE with
fused row-sum), and a column-shard of the output projection.
Host sums the two partial projections per batch.

Precision plan (validated vs reference on CPU, max-rel-err ~9e-3 < 2e-2):
- Q-proj, K-proj run in fp8e4 with DoubleRow perf mode (contraction 256 per
  instruction): x is converted to fp8 (x*8) on-chip, WQ/WK host-packed as
  fp8 (*512). The PSUM result carries a *4096 scale that rides the existing
  copies (k stays *4096 through the E-projection; q is rescaled to q*16 on
  the PSUM->SBUF copy into fp8 tiles).
- S = q @ kpT runs in fp8 DoubleRow too: q8 [128,2,TB] fp8 tiles pack a
  4-head quad along the 256-deep contraction; kpbp4 [128,2,512] fp8 tiles
  hold the paired kpT with zero-padding; the *256 net scale folds into the
  softmax exp scale.
- V-proj, E-projections, softmax, attention-out, and the final projection
  stay bf16/f32r.

Scheduling: phase 2 is Activation-bound (32 exp+accum instructions per
t-block, each paying the TRN2 access-latency bubble), so the Act engine runs
ONLY exps; all other copies go to DVE/GpSimd. The Q-projection runs
just-in-time inside phase 2 (x8 stays resident in SBUF) to shrink the
PE-bound phase 1. S matmuls are issued 2 units ahead of the PE transposes
that depend on the exp/normalize chain (no in-order head-of-line blocking),
and both heads' attention-out land in one [128,TB] PSUM tile via
partition-offset matmuls (no cross-partition DMA).
"""
import sys
for p in ('/opt/trn_rl_repo', '/root/.axon_site/_ro/trn_rl_repo'):
    if p not in sys.path:
        sys.path.insert(0, p)

from contextlib import ExitStack

import numpy as np

import concourse.bacc as bacc
import concourse.mybir as mybir
from concourse import tile
from concourse.bass_utils import run_bass_kernel_spmd

f32 = mybir.dt.float32
f32r = mybir.dt.float32r
bf16 = mybir.dt.bfloat16
f8 = mybir.dt.float8e4
AF = mybir.ActivationFunctionType
DR = mybir.MatmulPerfMode.DoubleRow

B, T, C = 4, 4096, 1024
H, HS = 16, 64
K = 256
HL = 8            # heads per core
TB = 512          # t-block
NTB = T // TB     # 8
NC_ = C // 128    # 8 c-chunks
SCALE = 1.0 / np.sqrt(np.float32(K))  # 1/16
# fp8 scales (powers of 2): x*8, W*512 -> psum *4096; q/kp tiles at *16.
XS = 8.0
WS = 512.0
QS = 16.0
RS = QS / (XS * WS)                   # psum -> *16 rescale (1/256)
EXP_SCALE = float(SCALE / (QS * QS))  # S psum carries q16*kp16 = 256*S_true


def to_f32r(a: np.ndarray) -> np.ndarray:
    """Round fp32 -> fp32r bit format (11-bit mantissa, low 12 bits zero), RNE."""
    b = np.ascontiguousarray(a, dtype=np.float32).view(np.uint32)
    add = np.uint32(0x7FF) + ((b >> np.uint32(12)) & np.uint32(1))
    return ((b + add) & np.uint32(0xFFFFF000)).view(np.float32)


def _build_program(phases=3, repeat=1, timing=False):
    nc = bacc.Bacc("TRN2", target_bir_lowering=False, debug=False, num_devices=8)

    if timing:
        DIN = nc.declare_dram_parameter("DIN", [128, 128], f32, isOutput=False)
        DOUT = nc.declare_dram_parameter("DOUT", [128, 128], f32, isOutput=True)
        decl = lambda name, shape, dt_, out=False: nc.dram_tensor(name, shape, dt_)
    else:
        decl = lambda name, shape, dt_, out=False: nc.declare_dram_parameter(
            name, shape, dt_, isOutput=out)
    XT = decl("XT", [C, T], bf16)
    WQ8 = decl("WQ8", [128, 4, 2, 512], f8)   # [p, cpair, i, m*128+md] = WQ[c,md]*512
    WK8 = decl("WK8", [128, 4, 2, 512], f8)
    WV = decl("WV", [128, NC_ * 512], bf16)   # host-packed: chunk c at cols c*512
    ED = decl("ED", [HL, T, K], bf16)
    WPT = decl("WPT", [128, 4 * C], f32r)     # host-packed: ci-chunk m at cols m*C
    MSK = decl("MSK", [2, 128, K], f32)
    IDN = decl("IDN", [128, 128], f32)
    O = decl("O", [T, C], f32, out=True)

    with tile.TileContext(nc) as tc, ExitStack() as top:
        # ---- persistent pools ----
        misc = top.enter_context(tc.tile_pool(name="misc", bufs=1))
        kvacc_p = top.enter_context(tc.tile_pool(name="kvacc", bufs=1))
        vp_p = top.enter_context(tc.tile_pool(name="vp", bufs=1))
        x8_p = top.enter_context(tc.tile_pool(name="x8", bufs=1))
        wp_p = top.enter_context(tc.tile_pool(name="wp", bufs=1))

        identf = misc.tile([128, 128], f32, tag="identf", name="identf")
        nc.sync.dma_start(identf[:], IDN[:])
        ident = misc.tile([128, 128], f32r, tag="ident", name="ident")
        nc.vector.tensor_copy(ident[:].bitcast(f32r), identf[:])
        identb = misc.tile([128, 128], bf16, tag="identb", name="identb")
        nc.scalar.copy(identb[:], identf[:])
        ones2 = misc.tile([128, 2], f32, tag="ones2", name="ones2")
        nc.gpsimd.memset(ones2[:], 1.0)
        masksb = []
        for i in range(2):
            mt = misc.tile([128, K], f32, tag=f"msk{i}", name=f"msk{i}")
            nc.sync.dma_start(mt[:], MSK[i])
            masksb.append(mt)

        # output projection weights tile (DMA deferred to end of phase 1 so
        # the startup SP queue serves phase-1 x/E tiles first)
        wpt = wp_p.tile([128, 4 * C], f32r, tag="wpt", name="wpt")

        if phases:
            kvacc = [kvacc_p.tile([128, K], f32, tag=f"kvacc{h}", name=f"kvacc{h}")
                     for h in range(HL)]
            vp_sb = [vp_p.tile([128, 128], bf16, tag=f"vp{h}", name=f"vp{h}")
                     for h in range(HL)]
            # S rhs: per pr (head pair), [128, 2, 2K] fp8 quad-packed, zero-padded
            kpbp4 = [vp_p.tile([128, 2, 2 * K], f8, tag=f"kpbp4_{pr}",
                               name=f"kpbp4_{pr}") for pr in range(4)]
            # x in fp8 (*8), resident for the phase-2 JIT Q-projection:
            # x8res[cp][p, i, t] = x[cp*256 + i*128 + p, t] * 8
            x8res = [x8_p.tile([128, 2, T], f8, tag=f"x8_{cp}", name=f"x8_{cp}")
                     for cp in range(4)]
            wq8 = wp_p.tile([128, 4, 2, 512], f8, tag="wq8", name="wq8")

        for _rep in range(max(1, repeat)):
            # ================= PHASE 1 =================
            if phases & 1:
                with ExitStack() as s1:
                    w_p = s1.enter_context(tc.tile_pool(name="w", bufs=1))
                    xt_p = s1.enter_context(tc.tile_pool(name="xt", bufs=10))
                    e_p = s1.enter_context(tc.tile_pool(name="e", bufs=10))
                    kv_p = s1.enter_context(tc.tile_pool(name="kv", bufs=2))
                    psk_p = s1.enter_context(tc.tile_pool(name="psk", bufs=2, space="PSUM"))
                    psv_p = s1.enter_context(tc.tile_pool(name="psv", bufs=2, space="PSUM"))
                    pskv_p = s1.enter_context(tc.tile_pool(name="pskv", bufs=2, space="PSUM"))

                    wk8 = w_p.tile([128, 4, 2, 512], f8, tag="wk8", name="wk8")
                    wvt = w_p.tile([128, NC_ * 512], bf16, tag="wvt", name="wvt")
                    nc.sync.dma_start(wk8[:], WK8[:])
                    nc.sync.dma_start(wvt[:], WV[:])

                    xtt = [None] * NC_
                    ett = [None] * HL
                    for tb in range(NTB):
                        t0 = tb * TB
                        tbo = tb % 2
                        if tbo == 0:
                            for c in range(NC_):
                                x_t = xt_p.tile([128, 2 * TB], bf16, tag="xt", name="xt")
                                nc.sync.dma_start(x_t[:], XT[c * 128:(c + 1) * 128,
                                                             t0:t0 + 2 * TB])
                                xtt[c] = x_t
                            for h in range(HL):
                                e_t = e_p.tile([128, 8, K], bf16, tag="et", name="et")
                                src = ED[h, t0:t0 + 2 * TB, :].rearrange(
                                    "(s p) r -> p s r", p=128)
                                nc.sync.dma_start(e_t[:], src)
                                ett[h] = e_t

                        # x -> fp8 (*8) into the resident tiles (DVE/Pool)
                        for cp in range(4):
                            dst0 = x8res[cp][:, 0, t0:t0 + TB]
                            dst1 = x8res[cp][:, 1, t0:t0 + TB]
                            src0 = xtt[2 * cp][:, tbo * TB:(tbo + 1) * TB]
                            src1 = xtt[2 * cp + 1][:, tbo * TB:(tbo + 1) * TB]
                            if cp % 2 == 0:
                                nc.vector.tensor_scalar_mul(dst0, src0, XS)
                                nc.gpsimd.tensor_scalar_mul(dst1, src1, XS)
                            else:
                                nc.gpsimd.tensor_scalar_mul(dst0, src0, XS)
                                nc.vector.tensor_scalar_mul(dst1, src1, XS)

                        # K (fp8 DR, keeps *4096 scale) and V (bf16) -> packed kv
                        # even head h: cols [h*128: k(64) | v(64)], odd head: [v | k]
                        kvsb = []
                        for sub in range(4):
                            psk = psk_p.tile([128, 512], f32, tag="psk", name="psk")
                            psv = psv_p.tile([128, 512], f32, tag="psv", name="psv")
                            for cp in range(4):
                                nc.tensor.matmul(psk[:],
                                                 x8res[cp][:, :, t0 + sub * 128:t0 + (sub + 1) * 128],
                                                 wk8[:, cp, :, :],
                                                 start=(cp == 0), stop=(cp == 3),
                                                 perf_mode=DR)
                            for c in range(NC_):
                                nc.tensor.matmul(psv[:],
                                                 xtt[c][:, tbo * TB + sub * 128:tbo * TB + (sub + 1) * 128],
                                                 wvt[:, c * 512:(c + 1) * 512],
                                                 start=(c == 0), stop=(c == NC_ - 1))
                            kvt = kv_p.tile([128, 1024], bf16, tag=f"kv{sub}", name=f"kv{sub}")
                            kv4 = kvt[:].rearrange("p (hp x s) -> p hp x s", hp=4, x=4, s=HS)
                            psk4 = psk[:].rearrange("p (hp e s) -> p hp e s", hp=4, e=2, s=HS)
                            psv4 = psv[:].rearrange("p (hp e s) -> p hp e s", hp=4, e=2, s=HS)
                            nc.scalar.copy(kv4[:, :, 0, :], psk4[:, :, 0, :])
                            nc.vector.tensor_copy(kv4[:, :, 1, :], psv4[:, :, 0, :])
                            nc.scalar.copy(kv4[:, :, 2, :], psv4[:, :, 1, :])
                            nc.vector.tensor_copy(kv4[:, :, 3, :], psk4[:, :, 1, :])
                            kvsb.append(kvt)

                        # kpT/vpT accumulation; even h -> rows [kpT*4096; vpT],
                        # odd h -> rows [vpT; kpT*4096]
                        for h in range(HL):
                            pskv = pskv_p.tile([128, K], f32, tag="pskv", name="pskv")
                            for sub in range(4):
                                nc.tensor.matmul(pskv[:], kvsb[sub][:, h * 128:(h + 1) * 128],
                                                 ett[h][:, tbo * 4 + sub, :],
                                                 start=(sub == 0), stop=(sub == 3))
                            if tb == 0:
                                nc.vector.tensor_copy(kvacc[h][:].bitcast(f32r), pskv[:])
                            else:
                                nc.vector.tensor_tensor(kvacc[h][:].bitcast(f32r), kvacc[h][:],
                                                        pskv[:], op=mybir.AluOpType.add)

                    # phase-2 weights: DMA'd here so the startup SP queue
                    # serves phase-1 x/E tiles first
                    if _rep == 0:
                        nc.sync.dma_start(wq8[:], WQ8[:])
                        nc.sync.dma_start(wpt[:], WPT[:])

                    # phase 1.5: vp via transpose (true scale); kpbp4 fp8 quad tiles
                    for pr in range(4):
                        nc.gpsimd.memset(kpbp4[pr][:], 0.0)
                    for h in range(HL):
                        pr, h01 = h // 2, h % 2
                        lo = (h01 == 0)   # kpT rows 0:64 if even head, else 64:128
                        j = pr % 2
                        if lo:
                            nc.gpsimd.tensor_scalar_mul(kpbp4[pr][0:HS, j, 0:K],
                                                        kvacc[h][0:HS, :], RS)
                        else:
                            nc.gpsimd.tensor_scalar_mul(kpbp4[pr][HS:128, j, K:2 * K],
                                                        kvacc[h][HS:128, :], RS)
                        for jj in range(2):
                            psvp = pskv_p.tile([128, K], f32, tag="pskv", name="psvp")
                            nc.tensor.transpose(psvp[:, 0:128].bitcast(f32r),
                                                kvacc[h][:, jj * 128:(jj + 1) * 128].bitcast(f32r),
                                                ident[:])
                            vcols = psvp[:, 64:128] if lo else psvp[:, 0:64]
                            if jj == 0:
                                nc.scalar.copy(vp_sb[h][:, 0:HS], vcols)
                            else:
                                nc.vector.tensor_copy(vp_sb[h][:, HS:2 * HS], vcols)

            # ================= PHASE 2 =================
            if phases & 2:
                with ExitStack() as s2:
                    q8_p = s2.enter_context(tc.tile_pool(name="q8", bufs=2))
                    ew_p = s2.enter_context(tc.tile_pool(name="ew", bufs=8))
                    wn_p = s2.enter_context(tc.tile_pool(name="wn", bufs=8))
                    zz_p = s2.enter_context(tc.tile_pool(name="zz", bufs=8))
                    wt_p = s2.enter_context(tc.tile_pool(name="wt", bufs=2))
                    at_p = s2.enter_context(tc.tile_pool(name="at", bufs=2))
                    out_p = s2.enter_context(tc.tile_pool(name="outp", bufs=3))
                    psq_p = s2.enter_context(tc.tile_pool(name="psq", bufs=1, space="PSUM"))
                    pss_p = s2.enter_context(tc.tile_pool(name="pss", bufs=2, space="PSUM"))
                    pswt_p = s2.enter_context(tc.tile_pool(name="pswt", bufs=1, space="PSUM"))
                    pso_p = s2.enter_context(tc.tile_pool(name="pso", bufs=1, space="PSUM"))
                    psp_p = s2.enter_context(tc.tile_pool(name="psp", bufs=2, space="PSUM"))

                    NU = NTB * 16          # 16 units (pr, tt) per t-block
                    LOOK = 2               # transpose lookahead (units)
                    wn_t = {}              # (pr, tt) -> (wn0, wn1)
                    pswt_t = {}            # pr -> (pswt0, pswt1)
                    attTs = {}             # tb -> [4 tiles]
                    q8_t = {}              # tb -> [2 tiles: quad a]

                    def emit_Q(tb):
                        """JIT Q-projection (fp8 DR) -> q8 tiles (q*16)."""
                        t0 = tb * TB
                        q8_t[tb] = [q8_p.tile([128, 2, TB], f8, tag=f"q8_{a}",
                                              name=f"q8_{a}") for a in range(2)]
                        for m in range(4):
                            psq = psq_p.tile([128, 512], f32, tag="psq", name="psq")
                            for cp in range(4):
                                nc.tensor.matmul(psq[:],
                                                 wq8[:, cp, :, m * 128:(m + 1) * 128],
                                                 x8res[cp][:, :, t0:t0 + TB],
                                                 start=(cp == 0), stop=(cp == 3),
                                                 perf_mode=DR)
                            dst = q8_t[tb][m // 2][:, m % 2, :]
                            nc.vector.tensor_scalar_mul(dst, psq[:], RS)

                    def emit_S(tb, pr, tt):
                        if pr == 0 and tt == 0:
                            attTs[tb] = [at_p.tile([128, TB], f32r, tag=f"attT{p}",
                                                   name=f"attT{p}") for p in range(4)]
                        a = pr // 2
                        pss = pss_p.tile([128, 2 * K], f32, tag="pss", name="pss")
                        tg = tb * 4 + tt   # global t chunk (128 rows)
                        nc.tensor.matmul(pss[:],
                                         q8_t[tb][a][:, :, tt * 128:(tt + 1) * 128],
                                         kpbp4[pr][:, :, :],
                                         start=True, stop=True, perf_mode=DR)
                        z = zz_p.tile([128, 2], f32, tag="z", name="z")
                        rec = zz_p.tile([128, 2], f32, tag="rec", name="rec")
                        exws = []
                        for h01 in range(2):
                            half = pss[:, h01 * K:(h01 + 1) * K]
                            if tg < 2:
                                nc.vector.tensor_tensor(half, half, masksb[tg][:],
                                                        op=mybir.AluOpType.add)
                            expw = ew_p.tile([128, K], f32, tag="expw", name="expw")
                            nc.scalar.activation(expw[:], half, AF.Exp,
                                                 scale=EXP_SCALE,
                                                 accum_out=z[:, h01:h01 + 1])
                            exws.append(expw)
                        nc.gpsimd.tensor_tensor(rec[:], ones2[:], z[:],
                                                op=mybir.AluOpType.divide)
                        wns = []
                        for h01 in range(2):
                            wn = wn_p.tile([128, K], bf16, tag="wn", name="wn")
                            nc.gpsimd.tensor_scalar_mul(wn[:], exws[h01][:],
                                                        rec[:, h01:h01 + 1])
                            wns.append(wn)
                        wn_t[(pr, tt)] = wns

                    def emit_T(tb, pr, tt):
                        if tt == 0:
                            pswt_t[pr] = tuple(
                                pswt_p.tile([128, TB], f32, tag=f"pswt{h01}",
                                            name=f"pswt{h01}") for h01 in range(2))
                        wns = wn_t.pop((pr, tt))
                        for h01 in range(2):
                            pswt = pswt_t[pr][h01]
                            wn = wns[h01]
                            nc.tensor.transpose(
                                pswt[:].bitcast(bf16)[:, tt * 128:(tt + 1) * 128],
                                wn[:, 0:128], identb[:])
                            nc.tensor.transpose(
                                pswt[:].bitcast(bf16)[:, TB + tt * 128:TB + (tt + 1) * 128],
                                wn[:, 128:256], identb[:])

                    def emit_pso(tb, pr):
                        psoT = pso_p.tile([128, TB], f32, tag="pso", name="pso")
                        for h01 in range(2):
                            h = 2 * pr + h01
                            wt = wt_p.tile([128, 2 * TB], bf16, tag=f"wt{h01}",
                                           name=f"wt{h01}")
                            pswt = pswt_t[pr][h01]
                            nc.vector.tensor_copy(wt[:], pswt[:].bitcast(bf16)[:])
                            dst = psoT[h01 * HS:(h01 + 1) * HS, :]
                            nc.tensor.matmul(dst, vp_sb[h][:, 0:HS], wt[:, 0:TB],
                                             start=True, stop=False)
                            nc.tensor.matmul(dst, vp_sb[h][:, HS:2 * HS], wt[:, TB:2 * TB],
                                             start=False, stop=True)
                        nc.vector.tensor_copy(attTs[tb][pr][:], psoT[:])

                    def emit_proj(tb):
                        t0 = tb * TB
                        att = attTs.pop(tb)
                        q8_t.pop(tb, None)
                        for sub in range(4):
                            outsb = out_p.tile([128, C], f32, tag="outsb", name="outsb")
                            for n in range(2):
                                psp = psp_p.tile([128, 512], f32, tag="psp", name="psp")
                                for ci in range(4):
                                    nc.tensor.matmul(psp[:],
                                                     att[ci][:, sub * 128:(sub + 1) * 128],
                                                     wpt[:, ci * C + n * 512:ci * C + (n + 1) * 512],
                                                     start=(ci == 0), stop=(ci == 3))
                                nc.vector.tensor_copy(outsb[:, n * 512:(n + 1) * 512],
                                                      psp[:])
                            row = t0 + sub * 128
                            nc.sync.dma_start(O[row:row + 128, :], outsb[:])

                    emit_Q(0)
                    for gu in range(NU + LOOK + 2):
                        if gu < NU:
                            tb, rem = divmod(gu, 16)
                            pr, tt = divmod(rem, 4)
                            emit_S(tb, pr, tt)
                        if LOOK <= gu < NU + LOOK:
                            v = gu - LOOK
                            tb2, rem2 = divmod(v, 16)
                            pr2, tt2 = divmod(rem2, 4)
                            emit_T(tb2, pr2, tt2)
                            if pr2 == 3 and tt2 == 1 and tb2 + 1 < NTB:
                                emit_Q(tb2 + 1)
                            if tt2 == 3:
                                emit_pso(tb2, pr2)
                        if gu >= LOOK + 2:
                            v3 = gu - LOOK - 2
                            if v3 % 16 == 15:
                                emit_proj(v3 // 16)

        if timing:
            dpool = top.enter_context(tc.tile_pool(name="dummy", bufs=1))
            dt_ = dpool.tile([128, 128], f32, tag="dummy", name="dummy")
            nc.sync.dma_start(dt_[:], DIN[:])
            nc.sync.dma_start(DOUT[:], dt_[:])

    nc.finalize()
    return nc


_NC_CACHE = {}


def _get_program(phases=3):
    if phases not in _NC_CACHE:
        _NC_CACHE[phases] = _build_program(phases)
    return _NC_CACHE[phases]


def _pack_w(w_core):
    """[C, 512] -> [128, 8*512] with chunk c at cols c*512."""
    return np.ascontiguousarray(
        w_core.reshape(NC_, 128, 512).transpose(1, 0, 2).reshape(128, NC_ * 512))


def _pack_w8(w_core):
    """[C, 512] f32 -> [128, 4, 2, 512] fp8e4 (*512), c = cp*256 + i*128 + p."""
    import ml_dtypes
    w8 = (np.ascontiguousarray(w_core, np.float32) * WS).astype(ml_dtypes.float8_e4m3)
    return np.ascontiguousarray(w8.reshape(4, 2, 128, 512).transpose(2, 0, 1, 3))


def _make_in_maps(x, WQ, WK, WV, E, Wp):
    import ml_dtypes
    xr = np.transpose(np.asarray(x), (0, 2, 1)).astype(ml_dtypes.bfloat16)  # [B, C, T]
    wq_full = np.transpose(np.asarray(WQ), (1, 0, 2)).astype(np.float32)
    wk_full = np.transpose(np.asarray(WK), (1, 0, 2)).astype(np.float32)
    wv_full = np.transpose(np.asarray(WV), (1, 0, 2)).astype(ml_dtypes.bfloat16)
    er = np.asarray(E).astype(ml_dtypes.bfloat16)                 # [H, B, T, K]
    wpt_full = to_f32r(np.ascontiguousarray(np.asarray(Wp).T))    # [C_in, C_out]

    msk = np.zeros((2, 128, K), np.float32)
    for i in range(2):
        t_idx = i * 128 + np.arange(128)[:, None]
        msk[i] = np.where(np.arange(K)[None, :] <= t_idx, 0.0, -1e30)
    idn = np.eye(128, dtype=np.float32)

    in_maps = []
    for core in range(8):
        b, g = core // 2, core % 2
        hs = slice(g * HL, (g + 1) * HL)
        wpt_core = wpt_full[g * 512:(g + 1) * 512, :]              # [512, 1024]
        wpt_packed = np.ascontiguousarray(
            wpt_core.reshape(4, 128, C).transpose(1, 0, 2).reshape(128, 4 * C))
        in_maps.append({
            "XT": np.ascontiguousarray(xr[b]),
            "WQ8": _pack_w8(np.ascontiguousarray(wq_full[:, hs, :]).reshape(C, HL * HS)),
            "WK8": _pack_w8(np.ascontiguousarray(wk_full[:, hs, :]).reshape(C, HL * HS)),
            "WV": _pack_w(np.ascontiguousarray(wv_full[:, hs, :]).reshape(C, HL * HS)),
            "ED": np.ascontiguousarray(er[hs, b]),
            "WPT": wpt_packed,
            "MSK": msk,
            "IDN": idn,
        })
    return in_maps


def _run(x, WQ, WK, WV, E, Wp, bp, trace=False):
    nc = _get_program()
    in_maps = _make_in_maps(x, WQ, WK, WV, E, Wp)
    kw = {}
    if trace:
        kw = dict(trace=True, trace_cores=[0])
    res = run_bass_kernel_spmd(nc, in_maps, list(range(8)), **kw)
    out = np.zeros((B, T, C), np.float32)
    for b in range(B):
        out[b] = res.results[2 * b]["O"] + res.results[2 * b + 1]["O"]
    out += np.asarray(bp, np.float32)[None, None, :]
    return out, res


def kernel(x, WQ, WK, WV, E, Wp, bp):
    out, _ = _run(x, WQ, WK, WV, E, Wp, bp, trace=False)
    return out


def kernel_traced(x, WQ, WK, WV, E, Wp, bp):
    out, res = _run(x, WQ, WK, WV, E, Wp, bp, trace=True)
    return out, res


# revision 13
# speedup vs baseline: 3.9671x; 3.9671x over previous
"""Trainium2 Bass kernel for nn_MultiHeadLinearAttention (Linformer attention).

B=4, T=4096, C=1024, H=16, HS=64, K=256.
Sharding: 8 cores = batch (4) x head-group (2 groups of 8 heads).
Per core: qkv projections, low-rank kpT/vpT = k^T E / v^T E accumulated over
T, masked softmax attention over the compressed dim (exp on ScalarE with
fused row-sum), and a column-shard of the output projection.
Host sums the two partial projections per batch.

Precision plan (validated vs reference on CPU, max-rel-err ~9e-3 < 2e-2):
- Q-proj, K-proj run in fp8e4 with DoubleRow perf mode (contraction 256 per
  instruction): x is converted to fp8 (x*8) on-chip, WQ/WK host-packed as
  fp8 (*512). The PSUM result carries a *4096 scale that rides the existing
  copies (k stays *4096 through the E-projection; q is rescaled to q*16 on
  the PSUM->SBUF copy into fp8 tiles).
- S = q @ kpT runs in fp8 DoubleRow too: q8 [128,2,TB] fp8 tiles pack a
  4-head quad along the 256-deep contraction; kpbp4 [128,2,512] fp8 tiles
  hold the paired kpT with zero-padding; the *256 net scale folds into the
  softmax exp scale.
- V-proj, E-projections, softmax, attention-out, and the final projection
  stay bf16/f32r.

Scheduling: phase 2 is Activation-bound (32 exp+accum instructions per
t-block, each paying the TRN2 access-latency bubble), so the Act engine runs
ONLY exps; all other copies go to DVE/GpSimd. The Q-projection runs
just-in-time inside phase 2 (x8 stays resident in SBUF) to shrink the
PE-bound phase 1. S matmuls are issued 2 units ahead of the PE transposes
that depend on the exp/normalize chain (no in-order head-of-line blocking),
and both heads' attention-out land in one [128,TB] PSUM tile via
partition-offset matmuls (no cross-partition DMA).
"""
import sys
for p in ('/opt/trn_rl_repo', '/root/.axon_site/_ro/trn_rl_repo'):
    if p not in sys.path:
        sys.path.insert(0, p)

from contextlib import ExitStack

import numpy as np

import concourse.bacc as bacc
import concourse.mybir as mybir
from concourse import tile
from concourse.bass_utils import run_bass_kernel_spmd

f32 = mybir.dt.float32
f32r = mybir.dt.float32r
bf16 = mybir.dt.bfloat16
f8 = mybir.dt.float8e4
AF = mybir.ActivationFunctionType
DR = mybir.MatmulPerfMode.DoubleRow

B, T, C = 4, 4096, 1024
H, HS = 16, 64
K = 256
HL = 8            # heads per core
TB = 512          # t-block
NTB = T // TB     # 8
NC_ = C // 128    # 8 c-chunks
SCALE = 1.0 / np.sqrt(np.float32(K))  # 1/16
# fp8 scales (powers of 2): x*8, W*512 -> psum *4096; q/kp tiles at *16.
XS = 8.0
WS = 512.0
QS = 16.0
RS = QS / (XS * WS)                   # psum -> *16 rescale (1/256)
EXP_SCALE = float(SCALE / (QS * QS))  # S psum carries q16*kp16 = 256*S_true


def to_f32r(a: np.ndarray) -> np.ndarray:
    """Round fp32 -> fp32r bit format (11-bit mantissa, low 12 bits zero), RNE."""
    b = np.ascontiguousarray(a, dtype=np.float32).view(np.uint32)
    add = np.uint32(0x7FF) + ((b >> np.uint32(12)) & np.uint32(1))
    return ((b + add) & np.uint32(0xFFFFF000)).view(np.float32)


def _build_program(phases=3, repeat=1, timing=False):
    nc = bacc.Bacc("TRN2", target_bir_lowering=False, debug=False, num_devices=8)

    if timing:
        DIN = nc.declare_dram_parameter("DIN", [128, 128], f32, isOutput=False)
        DOUT = nc.declare_dram_parameter("DOUT", [128, 128], f32, isOutput=True)
        decl = lambda name, shape, dt_, out=False: nc.dram_tensor(name, shape, dt_)
    else:
        decl = lambda name, shape, dt_, out=False: nc.declare_dram_parameter(
            name, shape, dt_, isOutput=out)
    XT = decl("XT", [C, T], bf16)
    XT8 = decl("XT8", [128, 4, 2, T], f8)     # [p, cp, i, t] = x[cp*256+i*128+p, t]*8
    WQ8 = decl("WQ8", [128, 4, 2, 512], f8)   # [p, cpair, i, m*128+md] = WQ[c,md]*512
    WK8 = decl("WK8", [128, 4, 2, 512], f8)
    WV = decl("WV", [128, NC_ * 512], bf16)   # host-packed: chunk c at cols c*512
    ED = decl("ED", [HL, T, K], bf16)
    WPT = decl("WPT", [128, 4 * C], f32r)     # host-packed: ci-chunk m at cols m*C
    MSK = decl("MSK", [2, 128, K], f32)
    IDN = decl("IDN", [128, 128], f32)
    O = decl("O", [T, C], f32, out=True)

    with tile.TileContext(nc) as tc, ExitStack() as top:
        # ---- persistent pools ----
        misc = top.enter_context(tc.tile_pool(name="misc", bufs=1))
        kvacc_p = top.enter_context(tc.tile_pool(name="kvacc", bufs=1))
        vp_p = top.enter_context(tc.tile_pool(name="vp", bufs=1))
        x8_p = top.enter_context(tc.tile_pool(name="x8", bufs=1))
        wp_p = top.enter_context(tc.tile_pool(name="wp", bufs=1))

        identf = misc.tile([128, 128], f32, tag="identf", name="identf")
        nc.sync.dma_start(identf[:], IDN[:])
        ident = misc.tile([128, 128], f32r, tag="ident", name="ident")
        nc.vector.tensor_copy(ident[:].bitcast(f32r), identf[:])
        identb = misc.tile([128, 128], bf16, tag="identb", name="identb")
        nc.scalar.copy(identb[:], identf[:])
        masksb = []
        for i in range(2):
            mt = misc.tile([128, K], f32, tag=f"msk{i}", name=f"msk{i}")
            nc.sync.dma_start(mt[:], MSK[i])
            masksb.append(mt)

        # output projection weights tile (DMA deferred to end of phase 1 so
        # the startup SP queue serves phase-1 x/E tiles first)
        wpt = wp_p.tile([128, 4 * C], f32r, tag="wpt", name="wpt")

        if phases:
            kvacc = [kvacc_p.tile([128, K], f32, tag=f"kvacc{h}", name=f"kvacc{h}")
                     for h in range(HL)]
            vp_sb = [vp_p.tile([128, 128], bf16, tag=f"vp{h}", name=f"vp{h}")
                     for h in range(HL)]
            # S rhs: per pr (head pair), [128, 2, 2K] fp8 quad-packed, zero-padded
            kpbp4 = [vp_p.tile([128, 2, 2 * K], f8, tag=f"kpbp4_{pr}",
                               name=f"kpbp4_{pr}") for pr in range(4)]
            # x in fp8 (*8), resident for the phase-2 JIT Q-projection:
            # x8res[cp][p, i, t] = x[cp*256 + i*128 + p, t] * 8
            x8res = [x8_p.tile([128, 2, T], f8, tag=f"x8_{cp}", name=f"x8_{cp}")
                     for cp in range(4)]
            wq8 = wp_p.tile([128, 4, 2, 512], f8, tag="wq8", name="wq8")

        for _rep in range(max(1, repeat)):
            # ================= PHASE 1 =================
            if phases & 1:
                with ExitStack() as s1:
                    w_p = s1.enter_context(tc.tile_pool(name="w", bufs=1))
                    xt_p = s1.enter_context(tc.tile_pool(name="xt", bufs=10))
                    e_p = s1.enter_context(tc.tile_pool(name="e", bufs=10))
                    kv_p = s1.enter_context(tc.tile_pool(name="kv", bufs=2))
                    psk_p = s1.enter_context(tc.tile_pool(name="psk", bufs=2, space="PSUM"))
                    psv_p = s1.enter_context(tc.tile_pool(name="psv", bufs=2, space="PSUM"))
                    pskv_p = s1.enter_context(tc.tile_pool(name="pskv", bufs=2, space="PSUM"))

                    wk8 = w_p.tile([128, 4, 2, 512], f8, tag="wk8", name="wk8")
                    wvt = w_p.tile([128, NC_ * 512], bf16, tag="wvt", name="wvt")
                    nc.sync.dma_start(wk8[:], WK8[:])
                    nc.sync.dma_start(wvt[:], WV[:])

                    xtt = [None] * NC_
                    ett = [None] * HL
                    for tb in range(NTB):
                        t0 = tb * TB
                        tbo = tb % 2
                        if tbo == 0:
                            for c in range(NC_):
                                x_t = xt_p.tile([128, 2 * TB], bf16, tag="xt", name="xt")
                                nc.sync.dma_start(x_t[:], XT[c * 128:(c + 1) * 128,
                                                             t0:t0 + 2 * TB])
                                xtt[c] = x_t
                            for cp in range(4):
                                nc.sync.dma_start(
                                    x8res[cp][:, :, t0:t0 + 2 * TB],
                                    XT8[:, cp, :, t0:t0 + 2 * TB])
                            for h in range(HL):
                                e_t = e_p.tile([128, 8, K], bf16, tag="et", name="et")
                                src = ED[h, t0:t0 + 2 * TB, :].rearrange(
                                    "(s p) r -> p s r", p=128)
                                nc.sync.dma_start(e_t[:], src)
                                ett[h] = e_t

                        # K (fp8 DR, keeps *4096 scale) and V (bf16) -> packed kv
                        # even head h: cols [h*128: k(64) | v(64)], odd head: [v | k]
                        kvsb = []
                        for sub in range(4):
                            psk = psk_p.tile([128, 512], f32, tag="psk", name="psk")
                            psv = psv_p.tile([128, 512], f32, tag="psv", name="psv")
                            for cp in range(4):
                                nc.tensor.matmul(psk[:],
                                                 x8res[cp][:, :, t0 + sub * 128:t0 + (sub + 1) * 128],
                                                 wk8[:, cp, :, :],
                                                 start=(cp == 0), stop=(cp == 3),
                                                 perf_mode=DR)
                            for c in range(NC_):
                                nc.tensor.matmul(psv[:],
                                                 xtt[c][:, tbo * TB + sub * 128:tbo * TB + (sub + 1) * 128],
                                                 wvt[:, c * 512:(c + 1) * 512],
                                                 start=(c == 0), stop=(c == NC_ - 1))
                            kvt = kv_p.tile([128, 1024], bf16, tag=f"kv{sub}", name=f"kv{sub}")
                            kv4 = kvt[:].rearrange("p (hp x s) -> p hp x s", hp=4, x=4, s=HS)
                            psk4 = psk[:].rearrange("p (hp e s) -> p hp e s", hp=4, e=2, s=HS)
                            psv4 = psv[:].rearrange("p (hp e s) -> p hp e s", hp=4, e=2, s=HS)
                            nc.scalar.copy(kv4[:, :, 0, :], psk4[:, :, 0, :])
                            nc.vector.tensor_copy(kv4[:, :, 1, :], psv4[:, :, 0, :])
                            nc.scalar.copy(kv4[:, :, 2, :], psv4[:, :, 1, :])
                            nc.vector.tensor_copy(kv4[:, :, 3, :], psk4[:, :, 1, :])
                            kvsb.append(kvt)

                        # kpT/vpT accumulation; even h -> rows [kpT*4096; vpT],
                        # odd h -> rows [vpT; kpT*4096]
                        for h in range(HL):
                            pskv = pskv_p.tile([128, K], f32, tag="pskv", name="pskv")
                            for sub in range(4):
                                nc.tensor.matmul(pskv[:], kvsb[sub][:, h * 128:(h + 1) * 128],
                                                 ett[h][:, tbo * 4 + sub, :],
                                                 start=(sub == 0), stop=(sub == 3))
                            if tb == 0:
                                nc.vector.tensor_copy(kvacc[h][:].bitcast(f32r), pskv[:])
                            else:
                                nc.vector.tensor_tensor(kvacc[h][:].bitcast(f32r), kvacc[h][:],
                                                        pskv[:], op=mybir.AluOpType.add)

                    # phase-2 weights: DMA'd here so the startup SP queue
                    # serves phase-1 x/E tiles first
                    if _rep == 0:
                        nc.sync.dma_start(wq8[:], WQ8[:])
                        nc.sync.dma_start(wpt[:], WPT[:])

                    # phase 1.5: vp via transpose (true scale); kpbp4 fp8 quad tiles
                    for pr in range(4):
                        nc.gpsimd.memset(kpbp4[pr][:], 0.0)
                    for h in range(HL):
                        pr, h01 = h // 2, h % 2
                        lo = (h01 == 0)   # kpT rows 0:64 if even head, else 64:128
                        j = pr % 2
                        if lo:
                            nc.scalar.mul(kpbp4[pr][0:HS, j, 0:K],
                                          kvacc[h][0:HS, :], RS)
                        else:
                            nc.vector.tensor_scalar_mul(kpbp4[pr][HS:128, j, K:2 * K],
                                                        kvacc[h][HS:128, :], RS)
                        for jj in range(2):
                            psvp = pskv_p.tile([128, K], f32, tag="pskv", name="psvp")
                            nc.tensor.transpose(psvp[:, 0:128].bitcast(f32r),
                                                kvacc[h][:, jj * 128:(jj + 1) * 128].bitcast(f32r),
                                                ident[:])
                            vcols = psvp[:, 64:128] if lo else psvp[:, 0:64]
                            if jj == 0:
                                nc.scalar.copy(vp_sb[h][:, 0:HS], vcols)
                            else:
                                nc.vector.tensor_copy(vp_sb[h][:, HS:2 * HS], vcols)

            # ================= PHASE 2 =================
            if phases & 2:
                with ExitStack() as s2:
                    q8_p = s2.enter_context(tc.tile_pool(name="q8", bufs=2))
                    ew_p = s2.enter_context(tc.tile_pool(name="ew", bufs=8))
                    wn_p = s2.enter_context(tc.tile_pool(name="wn", bufs=8))
                    zz_p = s2.enter_context(tc.tile_pool(name="zz", bufs=8))
                    wt_p = s2.enter_context(tc.tile_pool(name="wt", bufs=2))
                    at_p = s2.enter_context(tc.tile_pool(name="at", bufs=2))
                    out_p = s2.enter_context(tc.tile_pool(name="outp", bufs=3))
                    psq_p = s2.enter_context(tc.tile_pool(name="psq", bufs=1, space="PSUM"))
                    pss_p = s2.enter_context(tc.tile_pool(name="pss", bufs=2, space="PSUM"))
                    pswt_p = s2.enter_context(tc.tile_pool(name="pswt", bufs=1, space="PSUM"))
                    pso_p = s2.enter_context(tc.tile_pool(name="pso", bufs=1, space="PSUM"))
                    psp_p = s2.enter_context(tc.tile_pool(name="psp", bufs=2, space="PSUM"))

                    NU = NTB * 16          # 16 units (pr, tt) per t-block
                    LOOK = 2               # transpose lookahead (units)
                    wn_t = {}              # (pr, tt) -> (wn0, wn1)
                    pswt_t = {}            # pr -> (pswt0, pswt1)
                    attTs = {}             # tb -> [4 tiles]
                    q8_t = {}              # tb -> [2 tiles: quad a]

                    def emit_Q(tb):
                        """JIT Q-projection (fp8 DR) -> q8 tiles (q*16)."""
                        t0 = tb * TB
                        q8_t[tb] = [q8_p.tile([128, 2, TB], f8, tag=f"q8_{a}",
                                              name=f"q8_{a}") for a in range(2)]
                        for m in range(4):
                            psq = psq_p.tile([128, 512], f32, tag="psq", name="psq")
                            for cp in range(4):
                                nc.tensor.matmul(psq[:],
                                                 wq8[:, cp, :, m * 128:(m + 1) * 128],
                                                 x8res[cp][:, :, t0:t0 + TB],
                                                 start=(cp == 0), stop=(cp == 3),
                                                 perf_mode=DR)
                            dst = q8_t[tb][m // 2][:, m % 2, :]
                            nc.vector.tensor_scalar_mul(dst, psq[:], RS)

                    def emit_S(tb, pr, tt):
                        if pr == 0 and tt == 0:
                            attTs[tb] = [at_p.tile([128, TB], f32r, tag=f"attT{p}",
                                                   name=f"attT{p}") for p in range(4)]
                        a = pr // 2
                        pss = pss_p.tile([128, 2 * K], f32, tag="pss", name="pss")
                        tg = tb * 4 + tt   # global t chunk (128 rows)
                        nc.tensor.matmul(pss[:],
                                         q8_t[tb][a][:, :, tt * 128:(tt + 1) * 128],
                                         kpbp4[pr][:, :, :],
                                         start=True, stop=True, perf_mode=DR)
                        z = zz_p.tile([128, 2], f32, tag="z", name="z")
                        rec = zz_p.tile([128, 2], f32, tag="rec", name="rec")
                        exws = []
                        for h01 in range(2):
                            half = pss[:, h01 * K:(h01 + 1) * K]
                            if tg < 2:
                                nc.vector.tensor_tensor(half, half, masksb[tg][:],
                                                        op=mybir.AluOpType.add)
                            expw = ew_p.tile([128, K], bf16, tag="expw", name="expw")
                            nc.scalar.activation(expw[:], half, AF.Exp,
                                                 scale=EXP_SCALE,
                                                 accum_out=z[:, h01:h01 + 1])
                            exws.append(expw)
                        nc.vector.reciprocal(rec[:], z[:])
                        wns = []
                        for h01 in range(2):
                            wn = wn_p.tile([128, K], bf16, tag="wn", name="wn")
                            nc.vector.tensor_scalar_mul(wn[:], exws[h01][:],
                                                        rec[:, h01:h01 + 1])
                            wns.append(wn)
                        wn_t[(pr, tt)] = wns

                    def emit_T(tb, pr, tt):
                        if tt == 0:
                            pswt_t[pr] = tuple(
                                pswt_p.tile([128, TB], f32, tag=f"pswt{h01}",
                                            name=f"pswt{h01}") for h01 in range(2))
                        wns = wn_t.pop((pr, tt))
                        for h01 in range(2):
                            pswt = pswt_t[pr][h01]
                            wn = wns[h01]
                            nc.tensor.transpose(
                                pswt[:].bitcast(bf16)[:, tt * 128:(tt + 1) * 128],
                                wn[:, 0:128], identb[:])
                            nc.tensor.transpose(
                                pswt[:].bitcast(bf16)[:, TB + tt * 128:TB + (tt + 1) * 128],
                                wn[:, 128:256], identb[:])

                    def emit_pso(tb, pr):
                        psoT = pso_p.tile([128, TB], f32, tag="pso", name="pso")
                        for h01 in range(2):
                            h = 2 * pr + h01
                            wt = wt_p.tile([128, 2 * TB], bf16, tag=f"wt{h01}",
                                           name=f"wt{h01}")
                            pswt = pswt_t[pr][h01]
                            nc.vector.tensor_copy(wt[:], pswt[:].bitcast(bf16)[:])
                            dst = psoT[h01 * HS:(h01 + 1) * HS, :]
                            nc.tensor.matmul(dst, vp_sb[h][:, 0:HS], wt[:, 0:TB],
                                             start=True, stop=False)
                            nc.tensor.matmul(dst, vp_sb[h][:, HS:2 * HS], wt[:, TB:2 * TB],
                                             start=False, stop=True)
                        nc.scalar.copy(attTs[tb][pr][:], psoT[:])

                    def emit_proj(tb):
                        t0 = tb * TB
                        att = attTs.pop(tb)
                        q8_t.pop(tb, None)
                        for sub in range(4):
                            outsb = out_p.tile([128, C], f32, tag="outsb", name="outsb")
                            for n in range(2):
                                psp = psp_p.tile([128, 512], f32, tag="psp", name="psp")
                                for ci in range(4):
                                    nc.tensor.matmul(psp[:],
                                                     att[ci][:, sub * 128:(sub + 1) * 128],
                                                     wpt[:, ci * C + n * 512:ci * C + (n + 1) * 512],
                                                     start=(ci == 0), stop=(ci == 3))
                                nc.vector.tensor_copy(outsb[:, n * 512:(n + 1) * 512],
                                                      psp[:])
                            row = t0 + sub * 128
                            nc.sync.dma_start(O[row:row + 128, :], outsb[:])

                    emit_Q(0)
                    for gu in range(NU + LOOK + 2):
                        if gu < NU:
                            tb, rem = divmod(gu, 16)
                            pr, tt = divmod(rem, 4)
                            emit_S(tb, pr, tt)
                        if LOOK <= gu < NU + LOOK:
                            v = gu - LOOK
                            tb2, rem2 = divmod(v, 16)
                            pr2, tt2 = divmod(rem2, 4)
                            emit_T(tb2, pr2, tt2)
                            if pr2 == 3 and tt2 == 1 and tb2 + 1 < NTB:
                                emit_Q(tb2 + 1)
                            if tt2 == 3:
                                emit_pso(tb2, pr2)
                        if gu >= LOOK + 2:
                            v3 = gu - LOOK - 2
                            if v3 % 16 == 15:
                                emit_proj(v3 // 16)

        if timing:
            dpool = top.enter_context(tc.tile_pool(name="dummy", bufs=1))
            dt_ = dpool.tile([128, 128], f32, tag="dummy", name="dummy")
            nc.sync.dma_start(dt_[:], DIN[:])
            nc.sync.dma_start(DOUT[:], dt_[:])

    nc.finalize()
    return nc


_NC_CACHE = {}


def _get_program(phases=3):
    if phases not in _NC_CACHE:
        _NC_CACHE[phases] = _build_program(phases)
    return _NC_CACHE[phases]


def _pack_w(w_core):
    """[C, 512] -> [128, 8*512] with chunk c at cols c*512."""
    return np.ascontiguousarray(
        w_core.reshape(NC_, 128, 512).transpose(1, 0, 2).reshape(128, NC_ * 512))


def _pack_w8(w_core):
    """[C, 512] f32 -> [128, 4, 2, 512] fp8e4 (*512), c = cp*256 + i*128 + p."""
    import ml_dtypes
    w8 = (np.ascontiguousarray(w_core, np.float32) * WS).astype(ml_dtypes.float8_e4m3)
    return np.ascontiguousarray(w8.reshape(4, 2, 128, 512).transpose(2, 0, 1, 3))


def _make_in_maps(x, WQ, WK, WV, E, Wp):
    import ml_dtypes
    xf = np.transpose(np.asarray(x), (0, 2, 1)).astype(np.float32)          # [B, C, T]
    xr = xf.astype(ml_dtypes.bfloat16)
    x8 = (xf * XS).astype(ml_dtypes.float8_e4m3)                            # [B, C, T]
    x8 = np.ascontiguousarray(
        x8.reshape(B, 4, 2, 128, T).transpose(0, 3, 1, 2, 4))               # [B,128,4,2,T]
    wq_full = np.transpose(np.asarray(WQ), (1, 0, 2)).astype(np.float32)
    wk_full = np.transpose(np.asarray(WK), (1, 0, 2)).astype(np.float32)
    wv_full = np.transpose(np.asarray(WV), (1, 0, 2)).astype(ml_dtypes.bfloat16)
    er = np.asarray(E).astype(ml_dtypes.bfloat16)                 # [H, B, T, K]
    wpt_full = to_f32r(np.ascontiguousarray(np.asarray(Wp).T))    # [C_in, C_out]

    msk = np.zeros((2, 128, K), np.float32)
    for i in range(2):
        t_idx = i * 128 + np.arange(128)[:, None]
        msk[i] = np.where(np.arange(K)[None, :] <= t_idx, 0.0, -1e30)
    idn = np.eye(128, dtype=np.float32)

    in_maps = []
    for core in range(8):
        b, g = core // 2, core % 2
        hs = slice(g * HL, (g + 1) * HL)
        wpt_core = wpt_full[g * 512:(g + 1) * 512, :]              # [512, 1024]
        wpt_packed = np.ascontiguousarray(
            wpt_core.reshape(4, 128, C).transpose(1, 0, 2).reshape(128, 4 * C))
        in_maps.append({
            "XT": np.ascontiguousarray(xr[b]),
            "XT8": x8[b],
            "WQ8": _pack_w8(np.ascontiguousarray(wq_full[:, hs, :]).reshape(C, HL * HS)),
            "WK8": _pack_w8(np.ascontiguousarray(wk_full[:, hs, :]).reshape(C, HL * HS)),
            "WV": _pack_w(np.ascontiguousarray(wv_full[:, hs, :]).reshape(C, HL * HS)),
            "ED": np.ascontiguousarray(er[hs, b]),
            "WPT": wpt_packed,
            "MSK": msk,
            "IDN": idn,
        })
    return in_maps


def _run(x, WQ, WK, WV, E, Wp, bp, trace=False):
    nc = _get_program()
    in_maps = _make_in_maps(x, WQ, WK, WV, E, Wp)
    kw = {}
    if trace:
        kw = dict(trace=True, trace_cores=[0])
    res = run_bass_kernel_spmd(nc, in_maps, list(range(8)), **kw)
    out = np.zeros((B, T, C), np.float32)
    for b in range(B):
        out[b] = res.results[2 * b]["O"] + res.results[2 * b + 1]["O"]
    out += np.asarray(bp, np.float32)[None, None, :]
    return out, res


def kernel(x, WQ, WK, WV, E, Wp, bp):
    out, _ = _run(x, WQ, WK, WV, E, Wp, bp, trace=False)
    return out


def kernel_traced(x, WQ, WK, WV, E, Wp, bp):
    out, res = _run(x, WQ, WK, WV, E, Wp, bp, trace=True)
    return out, res
